# revision 16
# baseline (speedup 1.0000x reference)
"""Distributed Trainium2 (Bass) kernel for nn_AtomEmbedder (2-layer GCN + embed).

Strategy (8 NeuronCores, node-sharded):
  - Nodes padded to 50176 = 8 * 6272; core c owns dst rows [c*6272, (c+1)*6272).
  - h0 = relu(x @ We + be) computed feature-major per shard (no transposes).
  - Per GCN layer l:
      g = dis * (h @ Wl)  (node-major shard, dense matmuls on PE)
      AllGather g into THREE region tables (windows 0-12 / 13-24 / 25-48 of
        each core's shard) so the first gathers start after only the first
        small collective instead of a monolithic half-table one.
      edge scatter: for each 128-dst-node window, accumulate in PSUM
        sum_e g[src_e] via dma_gather (token stream sorted by window, three
        region streams) + one-hot matmul (S^T @ gathered), plus self-loop
        (identity matmul on own shard rows) and bias folded as a K=1 matmul
        of sqrt(deg) x b.
      epilogue: out = act(dis * psum) in one ScalarE op per window.
  - Layer-1 windows are transposed on PE into feature-major h1T so layer-2
    dense matmuls start immediately (hidden under layer-1 gathers).
  - A tiny dummy AllGather issues first to absorb the one-time multi-core
    rendezvous barrier (~45us) under the constant loads / embed phase.
  - Layer-2 region AllGathers trigger mid-way through layer-1's pass B
    (after windows 12 / 24 / 48) so the inter-layer gap disappears.

The token gather is Q7-descriptor-rate-bound (~7.5 ns/token) on the gpsimd
engine; everything else (PE matmuls, DVE one-hot builds, ACT epilogues,
dense DMA, AllGathers) is scheduled to hide underneath it.
"""

import numpy as np
import ml_dtypes

BF16 = ml_dtypes.bfloat16
N_NODES = 50000
N_EDGES = 300000
F_IN = 11
D = 256
NCORES = 8
NSH = 6272            # nodes per core (50176 total, padded)
NTAB = NCORES * NSH   # 50176
NWIN = NSH // 128     # 49 windows per core
# region r covers windows [RWIN[r], RWIN[r+1]) of every core's shard
RWIN = [0, 13, 25, 49]
NREG = 3
RSTART = [RWIN[r] * 128 for r in range(NREG)]          # row start in-core
RROWS = [(RWIN[r + 1] - RWIN[r]) * 128 for r in range(NREG)]  # 1664,1536,3072
CALL = 2048           # tokens per dma_gather call
P = 128

_CACHE = {}


def _win_region(w):
    return 0 if w < RWIN[1] else (1 if w < RWIN[2] else 2)


def _edge_plan(src, dst):
    """Build the SPMD-uniform token-stream / segment structure plus per-core
    index data. Returns (meta, per_core_arrays)."""
    src = src.astype(np.int64)
    dst = dst.astype(np.int64)
    core = dst // NSH
    dloc = dst % NSH
    win = dloc // 128
    rsrc = src % NSH
    csrc = src // NSH
    reg = np.digitize(rsrc, [RSTART[1], RSTART[2]])       # 0,1,2
    tokval = csrc * np.take(RROWS, reg) + (rsrc - np.take(RSTART, reg))

    # counts[c, w, r]
    counts = np.zeros((NCORES, NWIN, NREG), dtype=np.int64)
    np.add.at(counts, (core, win, reg), 1)
    gcnt = counts.max(axis=0)              # [NWIN, NREG] uniform per-window

    # stream layout per region: windows in order, gcnt tokens each
    streams = {}
    for r in range(NREG):
        lens = gcnt[:, r]
        total = int(lens.sum())
        ncalls = max(1, -(-total // CALL))
        padded = ncalls * CALL
        last = total - (ncalls - 1) * CALL
        call_sizes = [CALL] * (ncalls - 1) + [max(128, -(-last // 128) * 128)]
        win_start = np.zeros(NWIN + 1, dtype=np.int64)
        win_start[1:] = np.cumsum(lens)
        streams[r] = dict(lens=lens, total=total, ncalls=ncalls, padded=padded,
                          win_start=win_start, call_sizes=call_sizes)

    # segments: (region, chunk, window) for every chunk x window overlap
    segs = []          # list of (region, chunk_idx, window)
    win_segs = {r: [[] for _ in range(NWIN)] for r in range(NREG)}
    for r in range(NREG):
        ws = streams[r]["win_start"]
        nchunk = streams[r]["padded"] // 128
        for w in range(NWIN):
            a, b = int(ws[w]), int(ws[w + 1])
            if a == b:
                continue
            for ch in range(a // 128, (b - 1) // 128 + 1):
                win_segs[r][w].append(len(segs))
                segs.append((r, ch, w))
        streams[r]["nchunk"] = nchunk
    nseg = len(segs)

    # per-core data: token indices (int16, per region) and dstloc columns
    per_core = []
    for c in range(NCORES):
        m = core == c
        s_c, w_c, r_c, dl_c = tokval[m], win[m], reg[m], dloc[m]
        core_tok = {}
        for r in range(NREG):
            st = streams[r]
            tok = np.zeros(st["padded"], dtype=np.int16)   # pad -> row 0
            dstl = np.full(st["padded"], -999.0, dtype=np.float32)
            mr = r_c == r
            s_r, w_r, dl_r = s_c[mr], w_c[mr], dl_c[mr]
            # sort by (window, src row): consecutive gather descriptors hit
            # nearby table rows -> better HBM row-buffer locality
            order = np.lexsort((s_r, w_r))
            s_r, w_r, dl_r = s_r[order], w_r[order], dl_r[order]
            # place each window's tokens at its global window start
            cnts = np.bincount(w_r, minlength=NWIN)
            pos = st["win_start"][w_r] + (np.arange(len(w_r))
                                          - np.repeat(np.cumsum(cnts) - cnts, cnts))
            tok[pos] = s_r.astype(np.int16)
            dstl[pos] = dl_r.astype(np.float32)
            core_tok[r] = (tok, dstl)
        # wrap idx tensors: [128, ncols]; within call k (2048 tokens ->
        # 128 cols), token j -> idxs[j % 16, k*128 + j // 16]
        idx_w = {}
        for r in range(NREG):
            tok = core_tok[r][0]
            st = streams[r]
            cols = []
            for k in range(st["ncalls"]):
                blk = tok[k * CALL:(k + 1) * CALL].reshape(128, 16).T  # [16,128]
                cols.append(blk)
            w16 = np.concatenate(cols, axis=1)          # [16, ncalls*128]
            idx_w[r] = np.tile(w16, (8, 1)).copy()      # [128, ncalls*128]
        # dstloc tensor [128, nseg]: segment s=(r, ch, w) -> column of
        # dloc - w*128 for the chunk's 128 tokens
        dstloc = np.full((128, nseg), -999.0, dtype=np.float32)
        for si, (r, ch, w) in enumerate(segs):
            dstl = core_tok[r][1]
            col = dstl[ch * 128:(ch + 1) * 128] - w * 128
            col[col < -500] = -999.0
            dstloc[:, si] = col
        per_core.append(dict(idx_r=[idx_w[r] for r in range(NREG)],
                             dstloc=dstloc))

    meta = dict(streams=streams, segs=segs, win_segs=win_segs, nseg=nseg)
    return meta, per_core


def _build_program(meta):
    import concourse.bass as bass
    import concourse.bacc as bacc
    import concourse.tile as tile
    import concourse.mybir as mybir

    f32 = mybir.dt.float32
    bf = mybir.dt.bfloat16
    i16 = mybir.dt.int16
    AF = mybir.ActivationFunctionType

    streams = meta["streams"]
    nseg = meta["nseg"]
    segs = meta["segs"]
    win_segs = meta["win_segs"]
    ncalls = {r: streams[r]["ncalls"] for r in range(NREG)}
    idx_cols = {r: ncalls[r] * 128 for r in range(NREG)}

    nc = bacc.Bacc("TRN2", target_bir_lowering=False, debug=False,
                   num_devices=NCORES)

    # ---- external I/O (per-core shards) ----
    xT = nc.dram_tensor("xT", [F_IN, NSH], bf, kind="ExternalInput")
    We = nc.dram_tensor("We", [F_IN, D], bf, kind="ExternalInput")
    beW = nc.dram_tensor("beW", [128, 2], f32, kind="ExternalInput")
    W1 = nc.dram_tensor("W1", [D, D], bf, kind="ExternalInput")
    W2 = nc.dram_tensor("W2", [D, D], bf, kind="ExternalInput")
    b1 = nc.dram_tensor("b1", [1, D], bf, kind="ExternalInput")
    b2 = nc.dram_tensor("b2", [1, D], bf, kind="ExternalInput")
    disw = nc.dram_tensor("disw", [128, NWIN], f32, kind="ExternalInput")
    sqd = nc.dram_tensor("sqd", [1, NSH], bf, kind="ExternalInput")
    iota = nc.dram_tensor("iota", [128, 128], f32, kind="ExternalInput")
    ident = nc.dram_tensor("ident", [128, 128], f32, kind="ExternalInput")
    identb = nc.dram_tensor("identb", [128, 128], bf, kind="ExternalInput")
    idx_t = [nc.dram_tensor(f"idx_r{r}", [128, idx_cols[r]], i16,
                            kind="ExternalInput") for r in range(NREG)]
    dstloc = nc.dram_tensor("dstloc", [128, nseg], f32, kind="ExternalInput")
    out = nc.dram_tensor("out", [NSH, D], f32, kind="ExternalOutput")

    with tile.TileContext(nc) as tc:
        with (
            tc.tile_pool(name="const", bufs=1) as constp,
            tc.tile_pool(name="hT", bufs=1) as hTp,
            tc.tile_pool(name="dram", bufs=1, space="DRAM") as dramp,
            tc.tile_pool(name="g0", bufs=4) as g0p,
            tc.tile_pool(name="g1", bufs=3) as g1p,
            tc.tile_pool(name="g2", bufs=4) as g2p,
            tc.tile_pool(name="acc", bufs=6, space="PSUM") as accp,
            tc.tile_pool(name="tps", bufs=2, space="PSUM") as tpsp,
            tc.tile_pool(name="sg", bufs=3) as sgp,
            tc.tile_pool(name="ob", bufs=3) as obp,
            tc.tile_pool(name="sm", bufs=4) as smp,
        ):
            # ---- load constants ----
            xT_sb = constp.tile([F_IN, NSH], bf)
            We_sb = constp.tile([F_IN, D], bf)
            beW_sb = constp.tile([128, 2], f32)
            W1_sb = constp.tile([128, 2, D], bf)
            W2_sb = constp.tile([128, 2, D], bf)
            b1_sb = constp.tile([1, D], bf)
            b2_sb = constp.tile([1, D], bf)
            disw_sb = constp.tile([128, NWIN], f32)
            sqd_sb = constp.tile([1, NSH], bf)
            iota_sb = constp.tile([128, 128], f32)
            ident_sb = constp.tile([128, 128], f32)
            identb_sb = constp.tile([128, 128], bf)
            idx_sb = [constp.tile([128, idx_cols[r]], i16, name=f"idx_sb{r}")
                      for r in range(NREG)]
            dstloc_sb = constp.tile([128, nseg], f32)

            nc.sync.dma_start(We_sb[:], We[:])
            nc.sync.dma_start(beW_sb[:], beW[:])
            nc.sync.dma_start(xT_sb[:], xT[:])
            nc.sync.dma_start(W1_sb[:, 0, :], W1[0:128, :])
            nc.sync.dma_start(W1_sb[:, 1, :], W1[128:256, :])
            nc.sync.dma_start(disw_sb[:], disw[:])
            nc.sync.dma_start(idx_sb[0][:], idx_t[0][:])
            nc.sync.dma_start(dstloc_sb[:], dstloc[:])
            nc.sync.dma_start(iota_sb[:], iota[:])
            nc.sync.dma_start(sqd_sb[:], sqd[:])
            nc.sync.dma_start(identb_sb[:], identb[:])
            nc.sync.dma_start(b1_sb[:], b1[:])
            nc.sync.dma_start(idx_sb[1][:], idx_t[1][:])
            nc.sync.dma_start(idx_sb[2][:], idx_t[2][:])
            nc.sync.dma_start(W2_sb[:, 0, :], W2[0:128, :])
            nc.sync.dma_start(W2_sb[:, 1, :], W2[128:256, :])
            nc.sync.dma_start(b2_sb[:], b2[:])
            nc.sync.dma_start(ident_sb[:], ident[:])

            h0T = hTp.tile([128, 2, NSH], bf)
            h1T = hTp.tile([128, 2, NSH], bf)

            # ---- embed: h0T = relu(We^T x^T + be), feature-major ----
            slabs = [(s, min(s + 512, NSH)) for s in range(0, NSH, 512)]
            for (a, b) in slabs:
                for k in (0, 1):
                    ps = accp.tile([128, 512], f32, tag="acc")
                    nc.tensor.matmul(ps[:, :b - a], lhsT=We_sb[:, k * 128:(k + 1) * 128],
                                     rhs=xT_sb[:, a:b], start=True, stop=True)
                    nc.scalar.activation(h0T[:, k, a:b], ps[:, :b - a], AF.Relu,
                                         bias=beW_sb[:, k:k + 1], scale=1.0)

            cc_in = {}
            cc_out = {}
            for l in (1, 2):
                cc_in[l] = [dramp.tile([RROWS[r], D], bf, name=f"ccin{r}_{l}")
                            for r in range(NREG)]
                cc_out[l] = [dramp.tile([NCORES * RROWS[r], D], bf,
                                        name=f"ccout{r}_{l}", addr_space="Shared")
                             for r in range(NREG)]
            accA = hTp.tile([128, NWIN, D], bf, name="accA")

            def dense(l, w, hT, W_sb):
                """g tile for window w of layer l -> SBUF + DMA to cc_in[l]."""
                ps = accp.tile([128, D], f32, tag="acc", name=f"dps{l}_{w}")
                for k in (0, 1):
                    nc.tensor.matmul(ps[:], lhsT=hT[:, k, w * 128:(w + 1) * 128],
                                     rhs=W_sb[:, k, :], start=(k == 0), stop=(k == 1))
                gt = obp.tile([128, D], bf, tag="ob", name=f"g{l}_{w}")
                nc.scalar.activation(gt[:], ps[:], AF.Copy, bias=0.0,
                                     scale=disw_sb[:, w:w + 1])
                r = _win_region(w)
                ww = w - RWIN[r]
                nc.sync.dma_start(cc_in[l][r][ww * 128:(ww + 1) * 128, :], gt[:])

            def allgather(l, r):
                nc.gpsimd.collective_compute(
                    "AllGather", mybir.AluOpType.bypass,
                    replica_groups=[list(range(NCORES))],
                    ins=[cc_in[l][r][:]], outs=[cc_out[l][r][:]])

            for w in range(NWIN):
                dense(1, w, h0T, W1_sb)
                if w == RWIN[1] - 1:
                    allgather(1, 0)
                elif w == RWIN[2] - 1:
                    allgather(1, 1)
            allgather(1, 2)

            def edge_phase(l, b_sb, post_b, post_win_b=None):
                """Two-pass edge scatter for layer l.

                Pass A: psum = self + bias + region-0/1 segments -> accA (bf16).
                Pass B: psum = region-2 segments + I @ accA -> post_b(w, psum).
                post_win_b(w) fires after pass-B window w is fully emitted
                (used to trigger layer-2 AllGathers mid-stream)."""
                gt_tiles = {r: {} for r in range(NREG)}
                emitted = {r: 0 for r in range(NREG)}
                pool = {0: g0p, 1: g1p, 2: g2p}
                tag = {0: "g0", 1: "g1", 2: "g2"}
                sizes = {r: streams[r]["call_sizes"] for r in range(NREG)}

                def emit_call(r):
                    k = emitted[r]
                    nidx = sizes[r][k]
                    g = pool[r].tile([128, nidx // 128, D], bf, tag=tag[r],
                                     name=f"L{l}r{r}c{k}")
                    nc.gpsimd.dma_gather(
                        out_ap=g[:], in_ap=cc_out[l][r][:],
                        idxs_ap=idx_sb[r][:, k * 128:k * 128 + nidx // 16],
                        num_idxs=nidx, num_idxs_reg=nidx, elem_size=D,
                        single_packet=False)
                    gt_tiles[r][k] = g
                    emitted[r] += 1

                def need(r, w):
                    return max((segs[si][1] * 128 // CALL + 1
                                for si in win_segs[r][w]), default=0)

                def seg_mms(rlist, w, ps, first_start, last_stop):
                    lst = [(r, si) for r in rlist for si in win_segs[r][w]]
                    for j, (r, si) in enumerate(lst):
                        _, ch, _ = segs[si]
                        call_k, cj = ch * 128 // CALL, (ch * 128 % CALL) // 128
                        S = smp.tile([128, 128], bf, tag="sm", name=f"S{l}_{si}")
                        nc.vector.tensor_tensor(
                            out=S[:],
                            in0=dstloc_sb[:, si:si + 1].to_broadcast([128, 128]),
                            in1=iota_sb[:],
                            op=mybir.AluOpType.is_equal)
                        nc.tensor.matmul(ps[:], lhsT=S[:],
                                         rhs=gt_tiles[r][call_k][:, cj, :],
                                         start=(first_start and j == 0),
                                         stop=(last_stop and j == len(lst) - 1))
                    return len(lst)

                # ---- pass A: regions 0 and 1, interleaved call emission ----
                merged = []
                e0 = e1 = 0
                n0, n1 = ncalls[0], ncalls[1]
                while e0 < n0 or e1 < n1:
                    if e0 < n0 and (e0 <= e1 + 1 or e1 >= n1):
                        merged.append((0, e0)); e0 += 1
                    else:
                        merged.append((1, e1)); e1 += 1
                mpos = 0
                for w in range(NWIN):
                    while mpos < len(merged) and (emitted[0] < need(0, w)
                                                  or emitted[1] < need(1, w)):
                        emit_call(merged[mpos][0])
                        mpos += 1
                    ps = accp.tile([128, D], f32, tag="acc", name=f"pa{l}_{w}")
                    sgt = sgp.tile([128, D], bf, tag="sg", name=f"sg{l}_{w}")
                    r = _win_region(w)
                    ww = w - RWIN[r]
                    nc.sync.dma_start(sgt[:], cc_in[l][r][ww * 128:(ww + 1) * 128, :])
                    nc.tensor.matmul(ps[:], lhsT=identb_sb[:], rhs=sgt[:],
                                     start=True, stop=False)
                    nseg_w = len(win_segs[0][w]) + len(win_segs[1][w])
                    nc.tensor.matmul(ps[:], lhsT=sqd_sb[0:1, w * 128:(w + 1) * 128],
                                     rhs=b_sb[0:1, :], start=False,
                                     stop=nseg_w == 0)
                    seg_mms((0, 1), w, ps, False, last_stop=True)
                    nc.scalar.copy(accA[:, w, :], ps[:])
                while mpos < len(merged):
                    emit_call(merged[mpos][0])
                    mpos += 1
                # ---- pass B: region 2 ----
                for w in range(NWIN):
                    while emitted[2] < need(2, w):
                        emit_call(2)
                    ps = accp.tile([128, D], f32, tag="acc", name=f"pb{l}_{w}")
                    seg_mms((2,), w, ps, True, last_stop=False)
                    nc.tensor.matmul(ps[:], lhsT=identb_sb[:], rhs=accA[:, w, :],
                                     start=len(win_segs[2][w]) == 0, stop=True)
                    post_b(w, ps)
                    if post_win_b is not None:
                        post_win_b(w)
                while emitted[2] < ncalls[2]:
                    emit_call(2)

            # ---- layer 1: edge phase -> h1T (transposed) + dense2 ----
            def l1_post(w, ps):
                ot = obp.tile([128, D], f32, tag="ob", name=f"h1_{w}")
                nc.scalar.activation(ot[:], ps[:], AF.Relu, bias=0.0,
                                     scale=disw_sb[:, w:w + 1])
                for k in (0, 1):
                    tp = tpsp.tile([128, 128], f32, tag="tp", name=f"tp{w}_{k}")
                    nc.tensor.transpose(tp[:], ot[:, k * 128:(k + 1) * 128],
                                        ident_sb[:])
                    nc.scalar.copy(h1T[:, k, w * 128:(w + 1) * 128], tp[:])
                dense(2, w, h1T, W2_sb)

            def l1_post_win(w):
                if w == RWIN[1] - 1:
                    allgather(2, 0)
                elif w == RWIN[2] - 1:
                    allgather(2, 1)
                elif w == NWIN - 1:
                    allgather(2, 2)

            edge_phase(1, b1_sb, l1_post, l1_post_win)

            # ---- layer 2 ----
            def l2_post(w, ps):
                ot = obp.tile([128, D], f32, tag="ob", name=f"o_{w}")
                nc.scalar.activation(ot[:], ps[:], AF.Copy, bias=0.0,
                                     scale=disw_sb[:, w:w + 1])
                nc.sync.dma_start(out[w * 128:(w + 1) * 128, :], ot[:])

            edge_phase(2, b2_sb, l2_post)

    nc.compile()
    return nc


def _prep_inputs(x, edge_index, W_embed, b_embed, W1, b1, W2, b2):
    src0, dst0 = np.asarray(edge_index[0]).astype(np.int64), \
        np.asarray(edge_index[1]).astype(np.int64)

    newid = np.arange(NTAB, dtype=np.int64)
    src, dst = src0, dst0
    meta, per_core = _edge_plan(src, dst)

    deg_d = 1.0 + np.zeros(NTAB, dtype=np.float64)
    np.add.at(deg_d, dst, 1)
    deg = deg_d
    dis = (1.0 / np.sqrt(deg)).astype(np.float32)
    sq = np.sqrt(deg).astype(np.float32)

    xpad = np.zeros((NTAB, F_IN), dtype=np.float32)
    xpad[newid[:N_NODES]] = x
    xT_full = np.ascontiguousarray(xpad.T)            # [11, NTAB]

    beW = np.asarray(b_embed, dtype=np.float32).reshape(2, 128).T.copy()  # [128,2]
    iota = np.tile(np.arange(128, dtype=np.float32), (128, 1))
    ident = np.eye(128, dtype=np.float32)

    in_maps = []
    for c in range(NCORES):
        sl = slice(c * NSH, (c + 1) * NSH)
        disw = dis[sl].reshape(NWIN, 128).T.copy()    # [128, NWIN]
        im = {
            "xT": np.ascontiguousarray(xT_full[:, sl]).astype(BF16),
            "We": np.asarray(W_embed, dtype=np.float32).astype(BF16),
            "beW": beW,
            "W1": np.asarray(W1, dtype=np.float32).astype(BF16),
            "W2": np.asarray(W2, dtype=np.float32).astype(BF16),
            "b1": np.asarray(b1, dtype=np.float32).reshape(1, D).astype(BF16),
            "b2": np.asarray(b2, dtype=np.float32).reshape(1, D).astype(BF16),
            "disw": disw,
            "sqd": sq[sl].reshape(1, NSH).astype(BF16),
            "iota": iota,
            "ident": ident,
            "identb": ident.astype(BF16),
            "dstloc": per_core[c]["dstloc"],
        }
        for r in range(NREG):
            im[f"idx_r{r}"] = per_core[c]["idx_r"][r]
        in_maps.append(im)
    return meta, in_maps, newid


def kernel(x, edge_index, W_embed, b_embed, W1, b1, W2, b2, _trace=False):
    from concourse.bass_utils import run_bass_kernel_spmd

    meta, in_maps, newid = _prep_inputs(x, edge_index, W_embed, b_embed,
                                        W1, b1, W2, b2)
    key = tuple(tuple(meta["streams"][r]["lens"].tolist()) for r in range(NREG))
    if key not in _CACHE:
        _CACHE.clear()
        _CACHE[key] = _build_program(meta)
    nc = _CACHE[key]

    res = run_bass_kernel_spmd(nc, in_maps, core_ids=list(range(NCORES)),
                               trace=_trace)
    full = np.concatenate([res.results[c]["out"] for c in range(NCORES)], axis=0)
    kernel._last_exec_ns = res.exec_time_ns
    return full[newid[:N_NODES]].astype(np.float32)
